# revision 2
# baseline (speedup 1.0000x reference)
"""Trainium2 Bass kernel for a 2-layer ReLU RNN (batch_first) + linear head.

Problem shapes: B=256, T=512, I=512, H=1024, O=256 (fp32).
Sharding: data-parallel over batch across 8 NeuronCores (32 rows each);
weights replicated. No collectives.

Per-core design (all matmul operands bf16, fp32 PSUM accumulate):
  The per-step input GEMMs are FUSED into the recurrences as extra
  stationary k-tiles, so there are no separate projection phases and no
  DRAM intermediates:
    L0 step t: s0 = x_t @ W_ih0.T + h0 @ W_hh0.T      (12 k-tiles)
    L1 step t: s1 = h0_t @ W_ih1.T + h1 @ W_hh1.T     (16 k-tiles)

  s-matmuls run 4x column-tiled (tile_size 128x32): col-tile v computes
  the interleaved output column set {j : (j//32)%4 == v}, streaming the
  weight columns through a strided AP. This makes the PSUM layout
    ps[32v + b, 32w + j'] = s[b, 128w + 32v + j']
  which is exactly 32x32-block-transpose compatible: after a DVE
  tensor_add of the (identically interleaved) bias, ONE StreamTranspose
  of [128, 256] yields hT[jj, 32kt + b] = (s + bias)[b, 128kt + jj],
  and a scalar-engine Relu produces the next transposed state. The PE
  does no transposes and never switches tile modes.

  Emission per super-step u: [L0-u MMs][L0-u chain][L1-(u-1) MMs]
  [L1-(u-1) chain]; each chain (DVE add -> DVE transpose -> ACT relu)
  executes concurrently with the other layer's matmul window, so the PE
  streams matmuls nearly back-to-back for the whole kernel.

kernel(**inputs) takes the FULL unsharded inputs (keys as in reference
setup_inputs) and returns the FULL [256, 256] output.
"""

import ml_dtypes
import numpy as np

import concourse.bass as bass
import concourse.tile as tile
import concourse.mybir as mybir
from concourse import bacc
from concourse.bass_utils import run_bass_kernel_spmd

F32 = mybir.dt.float32
BF16 = mybir.dt.bfloat16

B_FULL, T_FULL, I_DIM, H, O = 256, 512, 512, 1024, 256
N_CORES = 8
BL = B_FULL // N_CORES  # 32 batch rows per core
KX = I_DIM // 128       # 4 k-tiles of the input dim
KH = H // 128           # 8 k-tiles of the hidden dim


def _emit_step_mms(nc, ps, stat_tiles, w_sb, n_k, tag):
    """One recurrence step's s-matmuls, 4x column-tiled.

    ps:   psum tile [128, 256] f32
    stat_tiles: list of n_k stationary APs [128, 32] (bf16)
    w_sb: weight SBUF tile [128, n_k_max*1024] bf16; k-block kb columns
          [kb*1024 + j] hold W[j, 128*kb + r] at row r.
    Col-tile v covers output columns {j : (j//32)%4 == v}; streamed as an
    AP [128, 8, 32] with column index 128*w + 32*v + j'.
    """
    for kb in range(n_k):
        blk = w_sb[:, kb * 1024 : (kb + 1) * 1024].rearrange(
            "p (w f j) -> p w f j", f=4, j=32
        )
        for v in range(4):
            nc.tensor.matmul(
                ps[32 * v : 32 * v + 32, :],
                stat_tiles[kb],
                blk[:, :, v : v + 1, :],
                start=(kb == 0),
                stop=(kb == n_k - 1),
                tile_position=(0, 32 * v),
                skip_group_check=True,
            )


def build_rnn(T):
    nc = bacc.Bacc("TRN2", target_bir_lowering=False, debug=False)

    xTb_d = nc.dram_tensor("xTb", [I_DIM, T * BL], BF16, kind="ExternalInput").ap()
    w0_d = nc.dram_tensor("w0cat", [128, (KX + KH) * H], BF16, kind="ExternalInput").ap()
    w1_d = nc.dram_tensor("w1cat", [128, (KH + KH) * H], BF16, kind="ExternalInput").ap()
    fcw_d = nc.dram_tensor("fcwT", [128, KH * O], BF16, kind="ExternalInput").ap()
    b0_d = nc.dram_tensor("bias0il", [128, 256], F32, kind="ExternalInput").ap()
    b1_d = nc.dram_tensor("bias1il", [128, 256], F32, kind="ExternalInput").ap()
    fcb_d = nc.dram_tensor("fcb", [BL, O], F32, kind="ExternalInput").ap()
    out_d = nc.dram_tensor("out", [BL, O], F32, kind="ExternalOutput").ap()

    PREF = 4  # xt DMA prefetch depth (steps ahead)

    with tile.TileContext(nc) as tc:
        with (
            tc.tile_pool(name="wpool", bufs=1) as wpool,
            tc.tile_pool(name="cpool", bufs=1) as cpool,
            tc.tile_pool(name="xt", bufs=PREF + 2) as xt_pool,
            tc.tile_pool(name="hT0", bufs=3) as hT0_pool,
            tc.tile_pool(name="hT1", bufs=3) as hT1_pool,
            tc.tile_pool(name="sb0", bufs=2) as sb0_pool,
            tc.tile_pool(name="sb1", bufs=2) as sb1_pool,
            tc.tile_pool(name="tr0", bufs=2) as tr0_pool,
            tc.tile_pool(name="tr1", bufs=2) as tr1_pool,
            tc.tile_pool(name="ps0", bufs=2, space="PSUM") as ps0_pool,
            tc.tile_pool(name="ps1", bufs=2, space="PSUM") as ps1_pool,
            tc.tile_pool(name="psh", bufs=1, space="PSUM") as psh_pool,
            tc.tile_pool(name="eout", bufs=1) as eo_pool,
        ):
            w0_sb = wpool.tile([128, (KX + KH) * H], BF16)
            w1_sb = wpool.tile([128, (KH + KH) * H], BF16)
            fcw_sb = wpool.tile([128, KH * O], BF16)
            b0_sb = cpool.tile([128, 256], F32)
            b1_sb = cpool.tile([128, 256], F32)
            fcb_sb = cpool.tile([BL, O], F32)
            nc.sync.dma_start(w0_sb[:], w0_d)
            nc.sync.dma_start(w1_sb[:], w1_d)
            nc.sync.dma_start(fcw_sb[:], fcw_d)
            nc.sync.dma_start(b0_sb[:], b0_d)
            nc.sync.dma_start(b1_sb[:], b1_d)
            nc.sync.dma_start(fcb_sb[:], fcb_d)

            xT_view = xTb_d.rearrange("(ki p) n -> p ki n", p=128)

            def emit_xt_dma(t):
                xt = xt_pool.tile([128, KX * BL], BF16, tag="xt")
                nc.sync.dma_start(
                    xt[:, :].rearrange("p (ki b) -> p ki b", ki=KX),
                    xT_view[:, :, t * BL : (t + 1) * BL],
                )
                return xt

            xt_tiles = {}
            for t in range(min(T, PREF)):
                xt_tiles[t] = emit_xt_dma(t)

            def chain(ps, b_sb, sb_pool, tr_pool, hT_pool, tag):
                """psum s -> +bias (DVE) -> 32x32 transpose (DVE) -> relu
                (ACT) -> next transposed state tile [128, 256] bf16."""
                sb = sb_pool.tile([128, 256], BF16, tag=f"sb{tag}")
                nc.vector.tensor_add(sb[:, :], ps[:, :], b_sb[:, :])
                tr = tr_pool.tile([128, 256], BF16, tag=f"tr{tag}")
                nc.vector.transpose(tr[:, :], sb[:, :])
                hT = hT_pool.tile([128, 256], BF16, tag=f"hT{tag}")
                nc.scalar.activation(
                    hT[:, :], tr[:, :], mybir.ActivationFunctionType.Relu
                )
                return hT

            hT0 = None  # transposed L0 state of the previous step
            hT1 = None
            h0T = {}    # L0 outputs pending consumption by L1

            for u in range(T):
                if u + PREF < T:
                    xt_tiles[u + PREF] = emit_xt_dma(u + PREF)

                # ---- L0 step u ----
                xt = xt_tiles.pop(u)
                stats = [xt[:, 32 * k : 32 * k + 32] for k in range(KX)]
                n_k = KX
                if hT0 is not None:
                    stats += [hT0[:, 32 * k : 32 * k + 32] for k in range(KH)]
                    n_k += KH
                ps0 = ps0_pool.tile([128, 256], F32, tag="ps0")
                _emit_step_mms(nc, ps0, stats, w0_sb, n_k, "L0")
                hT0 = chain(ps0, b0_sb, sb0_pool, tr0_pool, hT0_pool, "0")
                h0T[u] = hT0

                # ---- L1 step u-1 ----
                if u >= 1:
                    h0 = h0T.pop(u - 1)
                    stats = [h0[:, 32 * k : 32 * k + 32] for k in range(KH)]
                    n_k = KH
                    if hT1 is not None:
                        stats += [hT1[:, 32 * k : 32 * k + 32] for k in range(KH)]
                        n_k += KH
                    ps1 = ps1_pool.tile([128, 256], F32, tag="ps1")
                    _emit_step_mms(nc, ps1, stats, w1_sb, n_k, "L1")
                    hT1 = chain(ps1, b1_sb, sb1_pool, tr1_pool, hT1_pool, "1")

            # ---- L1 step T-1 ----
            h0 = h0T.pop(T - 1)
            stats = [h0[:, 32 * k : 32 * k + 32] for k in range(KH)]
            n_k = KH
            if hT1 is not None:
                stats += [hT1[:, 32 * k : 32 * k + 32] for k in range(KH)]
                n_k += KH
            ps1 = ps1_pool.tile([128, 256], F32, tag="ps1")
            _emit_step_mms(nc, ps1, stats, w1_sb, n_k, "L1")
            hT1 = chain(ps1, b1_sb, sb1_pool, tr1_pool, hT1_pool, "1")

            # ---- head: out = h1_last @ fc_w.T + fc_b ----
            hps = psh_pool.tile([BL, O], F32)
            for kb in range(KH):
                nc.tensor.matmul(
                    hps[:, :],
                    hT1[:, 32 * kb : 32 * kb + 32],
                    fcw_sb[:, kb * O : (kb + 1) * O],
                    start=(kb == 0),
                    stop=(kb == KH - 1),
                    tile_position=(0, 0),
                    skip_group_check=True,
                )
            eo = eo_pool.tile([BL, O], F32)
            nc.vector.tensor_add(eo[:, :], hps[:, :], fcb_sb[:, :])
            nc.sync.dma_start(out_d, eo[:, :])

    nc.compile()
    return nc


def _stackT(W, n_k):
    """[128, n_k*cols] bf16: [r, kb*cols + j] = W[j, 128*kb + r]."""
    cols = W.shape[0]
    WT = np.ascontiguousarray(np.asarray(W, np.float32).T)  # [in, out]
    out = np.empty((128, n_k * cols), np.float32)
    for k in range(n_k):
        out[:, k * cols : (k + 1) * cols] = WT[128 * k : 128 * (k + 1), :]
    return out.astype(ml_dtypes.bfloat16)


def _bias_il(b):
    """Interleaved bias [128, 256] f32: [32v+b', 32w+j'] = b[128w+32v+j']."""
    arr = np.asarray(b, np.float32).reshape(8, 4, 32)  # [w, v, j']
    out = np.empty((128, 256), np.float32)
    for v in range(4):
        row = np.ascontiguousarray(arr[:, v, :]).reshape(256)
        out[32 * v : 32 * v + 32, :] = row[None, :]
    return out


def _prep_core_inputs(inputs, T):
    f32 = np.float32
    w0cat = np.concatenate(
        [_stackT(np.asarray(inputs["W_ih0"], f32), KX),
         _stackT(np.asarray(inputs["W_hh0"], f32), KH)], axis=1)
    w1cat = np.concatenate(
        [_stackT(np.asarray(inputs["W_ih1"], f32), KH),
         _stackT(np.asarray(inputs["W_hh1"], f32), KH)], axis=1)
    shared = {
        "w0cat": np.ascontiguousarray(w0cat),
        "w1cat": np.ascontiguousarray(w1cat),
        "fcwT": _stackT(np.asarray(inputs["fc_w"], f32), KH),
        "bias0il": _bias_il(np.asarray(inputs["b_ih0"], f32)
                            + np.asarray(inputs["b_hh0"], f32)),
        "bias1il": _bias_il(np.asarray(inputs["b_ih1"], f32)
                            + np.asarray(inputs["b_hh1"], f32)),
        "fcb": np.tile(np.asarray(inputs["fc_b"], f32)[None, :], (BL, 1)),
    }
    x = np.asarray(inputs["input_data"], f32)  # [B, T, I]
    in_maps = []
    for c in range(N_CORES):
        xs = x[c * BL : (c + 1) * BL, :T, :]  # [BL, T, I]
        xT = np.ascontiguousarray(np.transpose(xs, (2, 1, 0))).reshape(
            I_DIM, T * BL).astype(ml_dtypes.bfloat16)
        in_maps.append(dict(shared, xTb=xT))
    return in_maps


def run(inputs, trace=False, trace_kwargs=None, T=None):
    if T is None:
        T = np.asarray(inputs["input_data"]).shape[1]
    nc = build_rnn(T)
    in_maps = _prep_core_inputs(inputs, T)
    res = run_bass_kernel_spmd(
        nc, in_maps, list(range(N_CORES)), trace=trace, **(trace_kwargs or {})
    )
    out = np.concatenate([res.results[c]["out"] for c in range(N_CORES)], axis=0)
    return out, res


def kernel(**inputs):
    return run(inputs)[0]
